# revision 9
# baseline (speedup 1.0000x reference)
"""GAT layer kernel for Trainium2, 8 NeuronCores, data-parallel over R=b*s.

Self-contained: takes full inputs, returns full output.

v3 design (per core, RC=6 replicas):
  - Projection on PE in fp16: h_aug = x_r @ [W | Ws | Wd] (h-major head
    layout). Per-node HBM row n (3328 B, bf16):
      [h r0..r5 (6*256, h-major) | (as4 ad4) r0..r5 (48) | pad]
    assembled fully in SBUF (hslab6) and written with ONE contiguous DMA.
    ad lives in the SBUF copy of the rows, so no separate ad_sb tile.
  - Edge phase chunked by dst-tile (125 dsts, dst-sorted slots padded to
    128-slot tiles). Per chunk the row gather is issued as a SWDGE
    prepare_only descriptor-gen (cheap, ~1.6us on GpSimd) plus a
    trigger_dma that carries the data deps — descgen and DMA overlap the
    compute instead of blocking GpSimd for the whole transfer.
  - z = as[src] (gathered) + ad[dst] (PE expand via transposed one-hot);
    p = exp(leaky_relu(z)); den = segsum(p) via one-hot matmul;
    denrec = 0.25/den applied in node space after aggregation.
  - msg = hg * p (DVE, bf16, p broadcast over the 64 channels of each
    (replica, head) block); num = segsum(msg) (PE one-hot, f32 PSUM).
  - out = head-sum(num * denrec) + bias, then DMA out.
"""

import math
import numpy as np
import ml_dtypes

B, S, N, F = 4, 12, 1000, 64
H, C = 4, 64
HC = H * C            # 256
R = B * S             # 48
NCORES = 8
RC = R // NCORES      # 6 replicas per core
NEG_SLOPE = 0.2
DTW = 125             # dst-tile width (8 tiles cover N=1000)
NDT = N // DTW        # 8
AC = RC * H           # 24 attention scalar columns (6r x 4h)
ASAD0 = RC * HC       # 1536: offset of the interleaved (as4 ad4) block
ROWW = 1664           # row width in bf16 elems (1536 h + 48 asad + pad) = 3328B

_CACHE = {}


# --------------------------------------------------------------------------
# host-side index preprocessing
# --------------------------------------------------------------------------
def _prep_edges(edge_index):
    src0 = np.asarray(edge_index[0], dtype=np.int64)
    dst0 = np.asarray(edge_index[1], dtype=np.int64)
    keep = src0 != dst0                      # PyG remove_self_loops + NEG_INF mask
    s_all = np.concatenate([src0[keep], np.arange(N, dtype=np.int64)])
    d_all = np.concatenate([dst0[keep], np.arange(N, dtype=np.int64)])
    order = np.argsort(d_all, kind="stable")
    s_all, d_all = s_all[order], d_all[order]

    # per dst-tile slot lists, each padded to a multiple of 128
    chunks = []
    for dt in range(NDT):
        lo, hi = dt * DTW, (dt + 1) * DTW
        m = (d_all >= lo) & (d_all < hi)
        ss, dd = s_all[m], d_all[m]
        cnt = len(ss)
        ntile = max(1, math.ceil(cnt / 128))
        pad = ntile * 128 - cnt
        ss = np.concatenate([ss, np.full(pad, 1000, np.int64)])   # pad -> row 1000
        dd = np.concatenate([dd, np.full(pad, lo, np.int64)])
        real = np.concatenate([np.ones(cnt, bool), np.zeros(pad, bool)])
        # one-hot [p, t, dlocal] and transposed [t, dlocal, p]
        oh = np.zeros((128, ntile, DTW), np.float32)
        for j in range(ntile * 128):
            if real[j]:
                oh[j % 128, j // 128, dd[j] - lo] = 1.0
        chunks.append(dict(ntile=ntile, src=ss, oh=oh.astype(ml_dtypes.bfloat16),
                           ohT=np.ascontiguousarray(
                               oh.transpose(2, 1, 0)).astype(ml_dtypes.bfloat16)))

    maxt = max(c["ntile"] for c in chunks)
    T = sum(c["ntile"] for c in chunks)
    # index tensor: per chunk, slots wrapped [16, slots/16], replicated to 128
    ihw = np.zeros((128, T * 8), np.int16)   # 128 slots = 8 idx columns
    oh_all = np.zeros((128, T, DTW), ml_dtypes.bfloat16)
    ohT_all = np.zeros((128, T, 128), ml_dtypes.bfloat16)
    t0 = 0
    for c in chunks:
        nt_, ss = c["ntile"], c["src"]
        ni = nt_ * 128
        a = np.zeros((16, ni // 16), np.int16)
        a[np.arange(ni) % 16, np.arange(ni) // 16] = ss.astype(np.int16)
        ihw[:, t0 * 8:(t0 + nt_) * 8] = np.tile(a, (8, 1))
        oh_all[:, t0:t0 + nt_, :] = c["oh"]
        ohT_all[:DTW, t0:t0 + nt_, :] = c["ohT"].transpose(0, 1, 2).reshape(
            DTW, nt_, 128)
        t0 += nt_
    return {
        "T": T, "maxt": maxt, "ntiles": [c["ntile"] for c in chunks],
        "oh": np.ascontiguousarray(oh_all.reshape(128, T * DTW)),
        "ohT": np.ascontiguousarray(ohT_all.reshape(128, T * 128)),
        "ih": ihw,
    }


def _prep_weights(W, att_src, att_dst):
    W = np.asarray(W, np.float32)
    Ws = np.zeros((F, H), np.float32)
    Wd = np.zeros((F, H), np.float32)
    for h in range(H):
        Ws[:, h] = W[:, h * C:(h + 1) * C] @ np.asarray(att_src, np.float32)[h]
        Wd[:, h] = W[:, h * C:(h + 1) * C] @ np.asarray(att_dst, np.float32)[h]
    return np.concatenate([W, Ws, Wd], axis=1).astype(np.float16)   # [64, 264]


def _make_in_maps(x, W, att_src, att_dst, bias, ed):
    waug = _prep_weights(W, att_src, att_dst)
    bias_slab = np.tile(np.asarray(bias, np.float32)[None, :],
                        (128, RC)).reshape(128, RC * F)
    xr = np.asarray(x, np.float32).reshape(R, N, F)
    in_maps = []
    for cidx in range(NCORES):
        xc = xr[cidx * RC:(cidx + 1) * RC]
        xT = np.ascontiguousarray(
            xc.transpose(2, 0, 1).reshape(F, RC * N)).astype(np.float16)
        in_maps.append({
            "xT": xT, "w_aug": waug, "oh": ed["oh"], "ohT": ed["ohT"],
            "ih": ed["ih"], "bias_slab": bias_slab,
        })
    return in_maps


# --------------------------------------------------------------------------
# device program
# --------------------------------------------------------------------------
def _build_program(ed):
    import concourse.bass as bass
    import concourse.mybir as mybir
    import concourse.tile as tile
    from concourse import bacc

    T, maxt = ed["T"], ed["maxt"]
    ntiles = ed["ntiles"]
    f32 = mybir.dt.float32
    f16 = mybir.dt.float16
    bf16 = mybir.dt.bfloat16
    i16 = mybir.dt.int16
    Alu = mybir.AluOpType
    Act = mybir.ActivationFunctionType

    nc = bacc.Bacc("TRN2", target_bir_lowering=False, debug=False,
                   enable_asserts=False, num_devices=NCORES)

    xT_d = nc.dram_tensor("xT", [F, RC * N], f16, kind="ExternalInput").ap()
    waug_d = nc.dram_tensor("w_aug", [F, 264], f16, kind="ExternalInput").ap()
    oh_d = nc.dram_tensor("oh", [128, T * DTW], bf16, kind="ExternalInput").ap()
    ohT_d = nc.dram_tensor("ohT", [128, T * 128], bf16, kind="ExternalInput").ap()
    ih_d = nc.dram_tensor("ih", [128, T * 8], i16, kind="ExternalInput").ap()
    bias_d = nc.dram_tensor("bias_slab", [128, RC * F], f32, kind="ExternalInput").ap()
    out_d = nc.dram_tensor("out", [RC, N, F], f32, kind="ExternalOutput").ap()

    with tile.TileContext(nc) as tc:
        with (
            tc.tile_pool(name="const", bufs=1) as constp,
            tc.tile_pool(name="dram", bufs=1, space="DRAM") as dramp,
            tc.tile_pool(name="edge", bufs=2) as edgep,
            tc.tile_pool(name="big", bufs=2) as bigp,
            tc.tile_pool(name="fin", bufs=2) as finp,
            tc.tile_pool(name="ppsum", bufs=3, space="PSUM") as ppsum,
            tc.tile_pool(name="npsum", bufs=2, space="PSUM") as npsum,
        ):
            h_hbm = dramp.tile([N + 1, ROWW], bf16)

            # ---- constants ----
            waug = constp.tile([F, 264], f16)
            nc.sync.dma_start(waug[:], waug_d)
            oh = constp.tile([128, T, DTW], bf16)
            nc.sync.dma_start(oh[:], oh_d.rearrange("p (t d) -> p t d", d=DTW))
            ohT = constp.tile([128, T, 128], bf16)
            nc.sync.dma_start(ohT[:], ohT_d.rearrange("p (t e) -> p t e", e=128))
            ih = constp.tile([128, T * 8], i16)
            nc.sync.dma_start(ih[:], ih_d)
            bias_sl = constp.tile([128, RC, F], f32)
            nc.sync.dma_start(bias_sl[:], bias_d.rearrange("p (r f) -> p r f", f=F))
            xall = constp.tile([F, RC * N], f16)
            nc.sync.dma_start(xall[:], xT_d)

            # pad row 1000: h-part zeros, asad-part -1000 => p == 0 for pads
            padrow = constp.tile([1, ROWW], bf16)
            nc.vector.memset(padrow[:], 0.0)
            nc.vector.memset(padrow[:, ASAD0:ASAD0 + 2 * AC], -1000.0)
            nc.sync.dma_start(h_hbm[N:N + 1, :], padrow[:])

            # ---- phase A: projection; assembles hslab6, ONE DMA to h_hbm ----
            # hslab6[d, a, :] is HBM row n = a*125 + d.
            hslab6 = constp.tile([DTW, NDT, ROWW], bf16)
            nc.vector.memset(hslab6[:, :, ASAD0 + 2 * AC:], 0.0)
            for r in range(RC):
                for a in range(NDT):
                    ps = ppsum.tile([DTW, 264], f32, tag="scratch")
                    nc.tensor.matmul(
                        out=ps[:], lhsT=xall[:, r * N + a * DTW:r * N + (a + 1) * DTW],
                        rhs=waug[:], start=True, stop=True)
                    nc.scalar.copy(out=hslab6[:, a, r * HC:(r + 1) * HC],
                                   in_=ps[:, 0:HC])
                    nc.vector.tensor_copy(
                        out=hslab6[:, a, ASAD0 + 8 * r:ASAD0 + 8 * r + 8],
                        in_=ps[:, HC:HC + 8])
            nc.sync.dma_start(
                h_hbm[0:N, :].rearrange("(a d) e -> d a e", d=DTW), hslab6[:])

            # ad columns of hslab6, viewed per dst-tile for the eps expand:
            # [125, r, h] strided (as4 ad4 interleave => ad at +4, stride 8)
            def ad_view(dt):
                return hslab6[0:DTW, dt, ASAD0:ASAD0 + 8 * RC].rearrange(
                    "d (r g) -> d r g", g=8)[:, :, 4:8]

            gsems = [nc.alloc_semaphore(f"gsem{k}") for k in range(NDT)]

            # ---- per dst-tile chunks ----
            t0 = 0
            for dt in range(NDT):
                nt_ = ntiles[dt]
                ni = nt_ * 128
                hg = bigp.tile([128, maxt, ROWW], bf16, tag="big")
                nc.gpsimd.dma_gather(
                    out_ap=hg[:, 0:nt_, :], in_ap=h_hbm[:],
                    idxs_ap=ih[:, t0 * 8:(t0 + nt_) * 8],
                    num_idxs=ni, num_idxs_reg=ni, elem_size=ROWW,
                    single_packet=False, prepare_only=True, sem=gsems[dt])
                nc.gpsimd.trigger_dma(count=None)
                # explicit DVE gate on the gather completion (the framework
                # auto-wait missed the first chunk's consumer)
                nc.vector.wait_ge(gsems[dt], 16)

                # ad expand: [128(e), nt_, 24] psum via transposed one-hot
                eps = ppsum.tile([128, maxt, AC], f32, tag="scratch", name="eps")
                adv = ad_view(dt)
                for t in range(nt_):
                    nc.tensor.matmul(out=eps[:, t, :], lhsT=ohT[0:DTW, t0 + t, :],
                                     rhs=adv, start=True, stop=True)
                # z = as + ad ; leaky relu ; exp
                asv = hg[:, 0:nt_, ASAD0:ASAD0 + 8 * RC].rearrange(
                    "p t (r g) -> p t r g", g=8)[:, :, :, 0:H]
                z = edgep.tile([128, maxt, RC, H], f32, tag="z")
                nc.vector.tensor_tensor(
                    out=z[:, 0:nt_, :, :], in0=asv,
                    in1=eps[:, 0:nt_, :].rearrange("p t (r h) -> p t r h", h=H),
                    op=Alu.add)
                nc.vector.scalar_tensor_tensor(
                    out=z[:, 0:nt_], in0=z[:, 0:nt_], scalar=NEG_SLOPE,
                    in1=z[:, 0:nt_], op0=Alu.mult, op1=Alu.max)
                p_bf = edgep.tile([128, maxt, AC], bf16, tag="p")
                nc.scalar.activation(
                    out=p_bf[:, 0:nt_, :],
                    in_=z[:, 0:nt_].rearrange("p t r h -> p t (r h)"),
                    func=Act.Exp)

                # den = segsum(p); denrec = 0.25/den
                den_ps = ppsum.tile([DTW, AC], f32, tag="scratch", name="den_ps")
                for t in range(nt_):
                    nc.tensor.matmul(out=den_ps[:], lhsT=oh[:, t0 + t, :],
                                     rhs=p_bf[:, t, :],
                                     start=(t == 0), stop=(t == nt_ - 1))
                denrec = edgep.tile([DTW, AC], f32, tag="denrec")
                nc.vector.reciprocal(out=denrec[:], in_=den_ps[:])
                nc.vector.tensor_scalar_mul(denrec[:], denrec[:], 0.25)

                # msg = hg * p (in-place, p broadcast over 64 chans of (r,h))
                HW2 = RC * HC // 2          # 768 cols per half (3 replicas)
                for half in range(2):
                    c0 = half * HW2
                    for r in range(3 * half, 3 * half + 3):
                        hgr = hg[:, 0:nt_, r * HC:(r + 1) * HC].rearrange(
                            "p t (h c) -> p t h c", h=H)
                        pb = p_bf[:, 0:nt_, 4 * r:4 * r + 4].rearrange(
                            "p t (h o) -> p t h o", o=1).to_broadcast(
                            [128, nt_, H, C])
                        nc.vector.tensor_tensor(out=hgr, in0=hgr, in1=pb,
                                                op=Alu.mult)
                    nps = npsum.tile([DTW, 3, HC], f32, tag="num")
                    npsf = nps[:].rearrange("d r e -> d (r e)")
                    for t in range(nt_):
                        nc.tensor.matmul(out=npsf[:, 0:512],
                                         lhsT=oh[:, t0 + t, :],
                                         rhs=hg[:, t, c0:c0 + 512],
                                         start=(t == 0), stop=(t == nt_ - 1))
                    for t in range(nt_):
                        nc.tensor.matmul(out=npsf[:, 512:768],
                                         lhsT=oh[:, t0 + t, :],
                                         rhs=hg[:, t, c0 + 512:c0 + 768],
                                         start=(t == 0), stop=(t == nt_ - 1))

                    # finalize: numn = num * denrec, head-sum, bias, DMA out
                    numn = finp.tile([DTW, 3, H, C], f32, tag="numn")
                    drb = denrec[:, half * 12:half * 12 + 12].rearrange(
                        "d (r h o) -> d r h o", h=H, o=1).to_broadcast(
                        [DTW, 3, H, C])
                    nc.vector.tensor_tensor(
                        out=numn[:],
                        in0=nps[:].rearrange("d r (h c) -> d r h c", h=H),
                        in1=drb, op=Alu.mult)
                    t1 = finp.tile([DTW, 3, C], f32, tag="t1")
                    t2 = finp.tile([DTW, 3, C], f32, tag="t2")
                    ob = finp.tile([DTW, 3, C], f32, tag="ob")
                    nc.vector.tensor_tensor(out=t1[:], in0=numn[:, :, 0, :],
                                            in1=numn[:, :, 1, :], op=Alu.add)
                    nc.vector.tensor_tensor(out=t2[:], in0=numn[:, :, 2, :],
                                            in1=numn[:, :, 3, :], op=Alu.add)
                    nc.vector.tensor_tensor(out=t1[:], in0=t1[:], in1=t2[:],
                                            op=Alu.add)
                    nc.vector.tensor_tensor(
                        out=ob[:], in0=t1[:],
                        in1=bias_sl[0:DTW, half * 3:half * 3 + 3, :], op=Alu.add)
                    nc.sync.dma_start(
                        out_d[half * 3:half * 3 + 3,
                              dt * DTW:(dt + 1) * DTW, :].rearrange(
                            "r d f -> d r f"), ob[:])
                t0 += nt_

    nc.compile()
    return nc


# --------------------------------------------------------------------------
# public entry point
# --------------------------------------------------------------------------
def kernel(x, edge_index, W, att_src, att_dst, bias):
    key = hash(np.asarray(edge_index).tobytes())
    if key not in _CACHE:
        ed = _prep_edges(edge_index)
        _CACHE[key] = (_build_program(ed), ed)
    nc, ed = _CACHE[key]

    in_maps = _make_in_maps(x, W, att_src, att_dst, bias, ed)
    from concourse import bass_utils
    res = bass_utils.run_bass_kernel_spmd(nc, in_maps, core_ids=list(range(NCORES)))
    outs = [res.results[c]["out"] for c in range(NCORES)]
    out = np.concatenate(outs, axis=0).reshape(B, S, N, F).astype(np.float32)
    return out


# revision 16
# speedup vs baseline: 1.2851x; 1.2851x over previous
"""GAT layer kernel for Trainium2, 8 NeuronCores, data-parallel over R=b*s.

Self-contained: takes full inputs, returns full output.

v4 design (per core, RC=6 replicas):
  - Projection on PE in fp16: per (node-tile a, replica r) one matmul
    x_tile @ [Wc | Ws | Wd] -> PSUM [125, 264], then ONE scalar-engine
    copy into the row slab: per-node HBM row n (3328 B bf16) is
      [r0: h256 (c-major) asad8] ... [r5: ...] pad      (6*264 = 1584)
    Rows of node-tile a are written right after the tile finishes, so the
    h-table write overlaps projection. The slab doubles as the ad table
    for the edge phase (no separate ad_sb).
  - Edge phase chunked by dst-tile (125 dsts, dst-sorted slots padded to
    128-slot tiles), each chunk split into TWO half-gathers so the
    gather->z->p->mult->num chain pipelines at depth 3 (hg bufs=3).
    Gathers are SWDGE prepare_only + trigger_dma: descriptor generation
    (~8 ns/idx on this silicon) overlaps DMA and compute.
  - z = as[src] (gathered) + ad[dst] (PE expand via transposed one-hot);
    p = exp(leaky_relu(z)); den = segsum(p) accumulated across both
    halves; denrec = 0.25/den applied in node space after aggregation.
  - msg = hg * p in-place (DVE 2x mode: c-major puts the 4 heads
    innermost so the broadcast operand has stride-1 runs).
  - num = segsum(msg) (PE one-hot, f32 PSUM [125, 1536] across halves);
    out = head-sum(num * denrec) + bias, then DMA out.
"""

import math
import numpy as np
import ml_dtypes

B, S, N, F = 4, 12, 1000, 64
H, C = 4, 64
HC = H * C            # 256
R = B * S             # 48
NCORES = 8
RC = R // NCORES      # 6 replicas per core
NEG_SLOPE = 0.2
DTW = 125             # dst-tile width (8 tiles cover N=1000)
NDT = N // DTW        # 8
AC = RC * H           # 24 attention scalar columns (6r x 4h)
RB = 264              # per-replica block: 256 h + 4 as + 4 ad
ROWW = 1664           # row width in bf16 elems (6*264 = 1584 + pad) = 3328B

_CACHE = {}


# --------------------------------------------------------------------------
# host-side index preprocessing
# --------------------------------------------------------------------------
def _prep_edges(edge_index):
    src0 = np.asarray(edge_index[0], dtype=np.int64)
    dst0 = np.asarray(edge_index[1], dtype=np.int64)
    keep = src0 != dst0                      # PyG remove_self_loops + NEG_INF mask
    s_all = np.concatenate([src0[keep], np.arange(N, dtype=np.int64)])
    d_all = np.concatenate([dst0[keep], np.arange(N, dtype=np.int64)])
    order = np.argsort(d_all, kind="stable")
    s_all, d_all = s_all[order], d_all[order]

    # per dst-tile slot lists, each padded to a multiple of 128
    chunks = []
    for dt in range(NDT):
        lo, hi = dt * DTW, (dt + 1) * DTW
        m = (d_all >= lo) & (d_all < hi)
        ss, dd = s_all[m], d_all[m]
        cnt = len(ss)
        ntile = max(2, math.ceil(cnt / 128))
        pad = ntile * 128 - cnt
        ss = np.concatenate([ss, np.full(pad, 1000, np.int64)])   # pad -> row 1000
        dd = np.concatenate([dd, np.full(pad, lo, np.int64)])
        real = np.concatenate([np.ones(cnt, bool), np.zeros(pad, bool)])
        # one-hot [p, t, dlocal] and transposed [t, dlocal, p]
        oh = np.zeros((128, ntile, DTW), np.float32)
        for j in range(ntile * 128):
            if real[j]:
                oh[j % 128, j // 128, dd[j] - lo] = 1.0
        chunks.append(dict(ntile=ntile, src=ss, oh=oh.astype(ml_dtypes.bfloat16),
                           ohT=np.ascontiguousarray(
                               oh.transpose(2, 1, 0)).astype(ml_dtypes.bfloat16)))

    T = sum(c["ntile"] for c in chunks)
    # index tensor: per chunk, slots wrapped [16, slots/16], replicated to 128
    ihw = np.zeros((128, T * 8), np.int16)   # 128 slots = 8 idx columns
    oh_all = np.zeros((128, T, DTW), ml_dtypes.bfloat16)
    ohT_all = np.zeros((128, T, 128), ml_dtypes.bfloat16)
    t0 = 0
    halves = []          # (t0, nt) per half-gather, two per chunk
    ntiles = []
    for c in chunks:
        nt_, ss = c["ntile"], c["src"]
        ni = nt_ * 128
        a = np.zeros((16, ni // 16), np.int16)
        a[np.arange(ni) % 16, np.arange(ni) // 16] = ss.astype(np.int16)
        ihw[:, t0 * 8:(t0 + nt_) * 8] = np.tile(a, (8, 1))
        oh_all[:, t0:t0 + nt_, :] = c["oh"]
        ohT_all[:DTW, t0:t0 + nt_, :] = c["ohT"].reshape(DTW, nt_, 128)
        h0 = (nt_ + 1) // 2
        halves.append([(t0, h0), (t0 + h0, nt_ - h0)])
        ntiles.append(nt_)
        t0 += nt_
    maxth = max(h[1] for hs in halves for h in hs)
    return {
        "T": T, "maxth": maxth, "ntiles": ntiles, "halves": halves,
        "oh": np.ascontiguousarray(oh_all.reshape(128, T * DTW)),
        "ohT": np.ascontiguousarray(ohT_all.reshape(128, T * 128)),
        "ih": ihw,
    }


def _prep_weights(W, att_src, att_dst):
    W = np.asarray(W, np.float32)
    Ws = np.zeros((F, H), np.float32)
    Wd = np.zeros((F, H), np.float32)
    for h in range(H):
        Ws[:, h] = W[:, h * C:(h + 1) * C] @ np.asarray(att_src, np.float32)[h]
        Wd[:, h] = W[:, h * C:(h + 1) * C] @ np.asarray(att_dst, np.float32)[h]
    # c-major head interleave: device col c*4+h = W col h*64+c
    Wc = np.empty_like(W)
    for h in range(H):
        Wc[:, np.arange(C) * H + h] = W[:, h * C:(h + 1) * C]
    return np.concatenate([Wc, Ws, Wd], axis=1).astype(np.float16)   # [64, 264]


def _make_in_maps(x, W, att_src, att_dst, bias, ed):
    waug = _prep_weights(W, att_src, att_dst)
    bias_slab = np.tile(np.asarray(bias, np.float32)[None, :],
                        (128, RC)).reshape(128, RC * F)
    xr = np.asarray(x, np.float32).reshape(R, N, F)
    in_maps = []
    for cidx in range(NCORES):
        xc = xr[cidx * RC:(cidx + 1) * RC]
        xT = np.ascontiguousarray(
            xc.transpose(2, 0, 1).reshape(F, RC * N)).astype(np.float16)
        in_maps.append({
            "xT": xT, "w_aug": waug, "oh": ed["oh"], "ohT": ed["ohT"],
            "ih": ed["ih"], "bias_slab": bias_slab,
        })
    return in_maps


# --------------------------------------------------------------------------
# device program
# --------------------------------------------------------------------------
def _build_program(ed):
    import concourse.bass as bass
    import concourse.mybir as mybir
    import concourse.tile as tile
    from concourse import bacc

    T, maxth = ed["T"], ed["maxth"]
    halves = ed["halves"]
    f32 = mybir.dt.float32
    f16 = mybir.dt.float16
    bf16 = mybir.dt.bfloat16
    i16 = mybir.dt.int16
    Alu = mybir.AluOpType
    Act = mybir.ActivationFunctionType

    nc = bacc.Bacc("TRN2", target_bir_lowering=False, debug=False,
                   enable_asserts=False, num_devices=NCORES)

    xT_d = nc.dram_tensor("xT", [F, RC * N], f16, kind="ExternalInput").ap()
    waug_d = nc.dram_tensor("w_aug", [F, RB], f16, kind="ExternalInput").ap()
    oh_d = nc.dram_tensor("oh", [128, T * DTW], bf16, kind="ExternalInput").ap()
    ohT_d = nc.dram_tensor("ohT", [128, T * 128], bf16, kind="ExternalInput").ap()
    ih_d = nc.dram_tensor("ih", [128, T * 8], i16, kind="ExternalInput").ap()
    bias_d = nc.dram_tensor("bias_slab", [128, RC * F], f32, kind="ExternalInput").ap()
    out_d = nc.dram_tensor("out", [RC, N, F], f32, kind="ExternalOutput").ap()

    with tile.TileContext(nc) as tc:
        with (
            tc.tile_pool(name="const", bufs=1) as constp,
            tc.tile_pool(name="dram", bufs=1, space="DRAM") as dramp,
            tc.tile_pool(name="edge", bufs=3) as edgep,
            tc.tile_pool(name="big", bufs=3) as bigp,
            tc.tile_pool(name="fin", bufs=2) as finp,
            tc.tile_pool(name="ppsum", bufs=2, space="PSUM") as ppsum,
            tc.tile_pool(name="dpsum", bufs=2, space="PSUM") as dpsum,
            tc.tile_pool(name="npsum", bufs=1, space="PSUM") as npsum,
        ):
            h_hbm = dramp.tile([N + 1, ROWW], bf16)

            # ---- constants ----
            waug = constp.tile([F, RB], f16)
            nc.sync.dma_start(waug[:], waug_d)
            oh = constp.tile([128, T, DTW], bf16)
            nc.sync.dma_start(oh[:], oh_d.rearrange("p (t d) -> p t d", d=DTW))
            ohT = constp.tile([128, T, 128], bf16)
            nc.sync.dma_start(ohT[:], ohT_d.rearrange("p (t e) -> p t e", e=128))
            ih = constp.tile([128, T * 8], i16)
            nc.sync.dma_start(ih[:], ih_d)
            bias_sl = constp.tile([128, RC, F], f32)
            nc.sync.dma_start(bias_sl[:], bias_d.rearrange("p (r f) -> p r f", f=F))
            xall = constp.tile([F, RC * N], f16)
            nc.sync.dma_start(xall[:], xT_d)

            # pad row 1000: h-part zeros, asad parts -1000 => p == 0 for pads
            padrow = constp.tile([1, ROWW], bf16)
            nc.vector.memset(padrow[:], 0.0)
            nc.vector.memset(
                padrow[:, 0:RC * RB].rearrange("p (r q) -> p r q", q=RB)[:, :, HC:RB],
                -1000.0)
            nc.sync.dma_start(h_hbm[N:N + 1, :], padrow[:])

            # ---- phase A: projection; per node-tile rows written early ----
            # hslab6[d, a, :] is HBM row n = a*125 + d.
            hslab6 = constp.tile([DTW, NDT, ROWW], bf16)
            nc.vector.memset(hslab6[:, :, RC * RB:], 0.0)
            for a in range(NDT):
                for r in range(RC):
                    ps = ppsum.tile([DTW, RB], f32, tag="scratch")
                    nc.tensor.matmul(
                        out=ps[:], lhsT=xall[:, r * N + a * DTW:r * N + (a + 1) * DTW],
                        rhs=waug[:], start=True, stop=True)
                    nc.scalar.copy(out=hslab6[:, a, r * RB:(r + 1) * RB], in_=ps[:])
                nc.sync.dma_start(h_hbm[a * DTW:(a + 1) * DTW, :], hslab6[:, a, :])

            gsems = [nc.alloc_semaphore(f"gsem{k}") for k in range(2 * NDT)]

            # ---- per dst-tile chunks, two half-gathers each ----
            for dt in range(NDT):
                den_ps = dpsum.tile([DTW, AC], f32, tag="den", name="den_ps")
                nps = npsum.tile([DTW, RC, HC], f32, tag="num")
                denrec = edgep.tile([DTW, AC], f32, tag="denrec")
                nt_all = ed["ntiles"][dt]
                tfirst = halves[dt][0][0]
                for hk, (t0, nt_) in enumerate(halves[dt]):
                    ni = nt_ * 128
                    hg = bigp.tile([128, maxth, ROWW], bf16, tag="big")
                    nc.gpsimd.dma_gather(
                        out_ap=hg[:, 0:nt_, :], in_ap=h_hbm[:],
                        idxs_ap=ih[:, t0 * 8:(t0 + nt_) * 8],
                        num_idxs=ni, num_idxs_reg=ni, elem_size=ROWW,
                        single_packet=False, prepare_only=True,
                        sem=gsems[2 * dt + hk])
                    nc.gpsimd.trigger_dma(count=None)
                    # explicit DVE gate on gather completion (framework
                    # auto-wait misses some first consumers)
                    nc.vector.wait_ge(gsems[2 * dt + hk], 16)

                    # ad expand: [128(e), nt_, 24] psum via transposed one-hot
                    eps = ppsum.tile([128, maxth, AC], f32, tag="scratch",
                                     name="eps")
                    adv = hslab6[0:DTW, dt, 0:RC * RB].rearrange(
                        "d (r q) -> d r q", q=RB)[:, :, HC + H:RB]
                    for t in range(nt_):
                        nc.tensor.matmul(out=eps[:, t, :],
                                         lhsT=ohT[0:DTW, t0 + t, :],
                                         rhs=adv, start=True, stop=True)
                    # z = as + ad ; leaky relu ; exp
                    asv = hg[:, 0:nt_, 0:RC * RB].rearrange(
                        "p t (r q) -> p t r q", q=RB)[:, :, :, HC:HC + H]
                    z = edgep.tile([128, maxth, RC, H], f32, tag="z")
                    nc.vector.tensor_tensor(
                        out=z[:, 0:nt_], in0=asv,
                        in1=eps[:, 0:nt_, :].rearrange("p t (r h) -> p t r h",
                                                       h=H),
                        op=Alu.add)
                    nc.vector.scalar_tensor_tensor(
                        out=z[:, 0:nt_], in0=z[:, 0:nt_], scalar=NEG_SLOPE,
                        in1=z[:, 0:nt_], op0=Alu.mult, op1=Alu.max)
                    p_bf = edgep.tile([128, maxth, AC], bf16, tag="p")
                    nc.scalar.activation(
                        out=p_bf[:, 0:nt_, :],
                        in_=z[:, 0:nt_].rearrange("p t r h -> p t (r h)"),
                        func=Act.Exp)

                    # den += segsum(p) over this half's tiles
                    for t in range(nt_):
                        nc.tensor.matmul(
                            out=den_ps[:], lhsT=oh[:, t0 + t, :],
                            rhs=p_bf[:, t, :],
                            start=(t0 + t == tfirst),
                            stop=(t0 + t == tfirst + nt_all - 1))

                    # msg = hg * p in-place (2x DVE: heads innermost)
                    for r in range(RC):
                        hgr = hg[:, 0:nt_, r * RB:r * RB + HC].rearrange(
                            "p t (c h) -> p t c h", h=H)
                        pb = p_bf[:, 0:nt_, 4 * r:4 * r + 4].rearrange(
                            "p t (o h) -> p t o h", o=1).to_broadcast(
                            [128, nt_, C, H])
                        nc.vector.tensor_tensor(out=hgr, in0=hgr, in1=pb,
                                                op=Alu.mult)
                    # num += segsum(msg): 3 bank-aligned groups of 2 replicas
                    npsf = nps[:].rearrange("d r e -> d (r e)")
                    hgv = hg[:, 0:nt_, 0:RC * RB].rearrange(
                        "p t (r q) -> p t r q", q=RB)[:, :, :, 0:HC]
                    for g in range(3):
                        for t in range(nt_):
                            nc.tensor.matmul(
                                out=npsf[:, g * 512:(g + 1) * 512],
                                lhsT=oh[:, t0 + t, :],
                                rhs=hgv[:, t, 2 * g:2 * g + 2, :],
                                start=(t0 + t == tfirst),
                                stop=(t0 + t == tfirst + nt_all - 1))

                # denrec = 0.25/den
                nc.vector.reciprocal(out=denrec[:], in_=den_ps[:])
                nc.vector.tensor_scalar_mul(denrec[:], denrec[:], 0.25)

                # finalize per col-half: numn = num*denrec, head-sum, +bias
                for ch in range(2):
                    numn = finp.tile([DTW, 3, C, H], f32, tag="numn")
                    drb = denrec[:, ch * 12:ch * 12 + 12].rearrange(
                        "d (r o h) -> d r o h", h=H, o=1).to_broadcast(
                        [DTW, 3, C, H])
                    nc.vector.tensor_tensor(
                        out=numn[:],
                        in0=nps[:, 3 * ch:3 * ch + 3, :].rearrange(
                            "d r (c h) -> d r c h", h=H),
                        in1=drb, op=Alu.mult)
                    t1 = finp.tile([DTW, 3, C], f32, tag="t1")
                    t2 = finp.tile([DTW, 3, C], f32, tag="t2")
                    ob = finp.tile([DTW, 3, C], f32, tag="ob")
                    nc.vector.tensor_tensor(out=t1[:], in0=numn[:, :, :, 0],
                                            in1=numn[:, :, :, 1], op=Alu.add)
                    nc.vector.tensor_tensor(out=t2[:], in0=numn[:, :, :, 2],
                                            in1=numn[:, :, :, 3], op=Alu.add)
                    nc.vector.tensor_tensor(out=t1[:], in0=t1[:], in1=t2[:],
                                            op=Alu.add)
                    nc.vector.tensor_tensor(
                        out=ob[:], in0=t1[:],
                        in1=bias_sl[0:DTW, ch * 3:ch * 3 + 3, :], op=Alu.add)
                    nc.sync.dma_start(
                        out_d[ch * 3:ch * 3 + 3,
                              dt * DTW:(dt + 1) * DTW, :].rearrange(
                            "r d f -> d r f"), ob[:])

    nc.compile()
    return nc


# --------------------------------------------------------------------------
# public entry point
# --------------------------------------------------------------------------
def kernel(x, edge_index, W, att_src, att_dst, bias):
    key = hash(np.asarray(edge_index).tobytes())
    if key not in _CACHE:
        ed = _prep_edges(edge_index)
        _CACHE[key] = (_build_program(ed), ed)
    nc, ed = _CACHE[key]

    in_maps = _make_in_maps(x, W, att_src, att_dst, bias, ed)
    from concourse import bass_utils
    res = bass_utils.run_bass_kernel_spmd(nc, in_maps, core_ids=list(range(NCORES)))
    outs = [res.results[c]["out"] for c in range(NCORES)]
    out = np.concatenate(outs, axis=0).reshape(B, S, N, F).astype(np.float32)
    return out
